# revision 24
# baseline (speedup 1.0000x reference)
"""Multi-head causal attention (B=2, T=2048, C=1024, H=16) on 8 Trainium2
NeuronCores, tensor-parallel over heads (2 heads per core).

v4 — DMA-layout + schedule tuning on top of the v2/v3 restructure:
  - Host pre-packs x, w_qkv, and the y output into the exact SBUF layouts
    so every DMA moves 8-16KB contiguous per partition (descriptor-bound
    2KB lines were capping a queue at ~79 GB/s; packed lines run at HBM
    rate). All four x pair-loads are issued up front, split across the
    two HWDGE queues (sync + scalar).
  - Causal masks and the vaug ones-columns are generated on-device
    (gpsimd memset + affine_select) — no mask DMA at all.
  - Phase A (QKV projection) runs on chunk PAIRS: one [128, 8192] x
    tile, drains batched to [128, 1024], V transposed per 128-token
    block on the PE (bf16 identity transpose) into the [V_h0|1|V_h1|1]
    vaug layout with one strided DVE copy.
  - Phase B per (batch, q-chunk): both heads' scores in one 2-bank PSUM
    tile [128, 1024]; ONE exp activation for both heads; multiplicative
    {0,1} bf16 mask AFTER the exp (DVE 2x) on diagonal k-tiles only; AV
    keeps the ones-column trick (row 64 = softmax denominator).
  - Normalize: one [33,128]-selector fp32 matmul broadcasts both heads'
    denominators (h1 to partitions 64-127), one fast reciprocal, two DVE
    muls write a stacked [128, 512] ot tile (h1 write crosses partitions
    0-63 -> 64-127). Out-projection contracts all 128 partitions in one
    matmul per m-tile; one packed y DMA per chunk (bf16).
  - Software pipelining: each chunk's finish stages (rowsum copy, bc
    matmul, reciprocal+muls, out-projection) are emitted at k-tiles 0-3
    of the NEXT chunk; batch-1 QKV pairs are interleaved between batch-0
    attention chunks as PE filler.

Matmuls in bf16 (fp32 PSUM); softmax normalization in fp32.
"""

import os
import sys

for _p in ("/opt/trn_rl_repo", "/root/.axon_site/_ro/trn_rl_repo"):
    if os.path.isdir(_p) and _p not in sys.path:
        sys.path.insert(0, _p)

import ml_dtypes
import numpy as np

import concourse.bacc as bacc
import concourse.bass as bass
import concourse.mybir as mybir
import concourse.tile as tile
from concourse.bass_utils import run_bass_kernel_spmd
from concourse.masks import make_identity

B, T, C, H, D = 2, 2048, 1024, 16, 64
NCORES = 8
BT = B * T                      # 4096 flattened tokens
TC = 512                        # token chunk (matmul free dim)
NTC = BT // TC                  # 8 token chunks
FP = mybir.dt.float32
FPR = mybir.dt.float32r
BF = mybir.dt.bfloat16
ACT = mybir.ActivationFunctionType
AV_DELAY = 3                    # k-tiles the AV matmul trails the scores

LAST_RESULTS = None             # stashed BassKernelResults for test harness


def build_nc():
    nc = bacc.Bacc(None, target_bir_lowering=False, debug=False)

    # xp[pair*128 + p, k*1024 + q] = x[token, cin] with cin = k*128 + p,
    # token = pair*1024 + q  (SBUF layout, 16KB contiguous per partition)
    xp = nc.declare_dram_parameter("xp", [512, 8 * 1024], BF, isOutput=False)
    # wcp[p, k*384 + g*128 + c] = w_qkv[k*128 + p, g*1024 + core*128 + c]
    wcp = nc.declare_dram_parameter("wcp", [128, 3072], BF, isOutput=False)
    wout = nc.declare_dram_parameter("wout", [128, C], BF, isOutput=False)
    bqkv = nc.declare_dram_parameter("bqkv", [128, 3], FP, isOutput=False)
    # [33, 128] selector: row 0 -> output partitions 0-63 (head 0), row 32
    # -> partitions 64-127 (head 1), rows 1-31 zero (engine partition
    # bases must be multiples of 32). One fp32 matmul broadcasts both
    # heads' softmax denominators.
    onesr = nc.declare_dram_parameter("onesr", [33, 128], FP, isOutput=False)
    # ypt[tcx*128 + p, m*512 + q] = y_partial[m*128 + p, tcx*512 + q]
    ypt = nc.declare_dram_parameter("ypt", [NTC * 128, 8 * TC], BF,
                                    isOutput=True)

    with tile.TileContext(nc) as tc:
        with (
            tc.tile_pool(name="const", bufs=1) as cpool,
            tc.tile_pool(name="big", bufs=1) as bigpool,
            tc.tile_pool(name="sb", bufs=2) as sbpool,
            tc.tile_pool(name="ps", bufs=2, space="PSUM") as pspool,
        ):
            # ---- x prefetch: all 4 pairs up front, split across queues ----
            wc_sb = cpool.tile([128, 3072], BF)
            nc.sync.dma_start(out=wc_sb[:], in_=wcp[:, :])
            xtiles = []
            for pair in range(4):
                xtile = sbpool.tile([128, 8 * 1024], BF, tag="xt", bufs=4)
                src = xp[pair * 128:(pair + 1) * 128, :]
                nc.sync.dma_start(out=xtile[:, 0:4096], in_=src[:, 0:4096])
                nc.scalar.dma_start(out=xtile[:, 4096:8192],
                                    in_=src[:, 4096:8192])
                xtiles.append(xtile)

            # ---- small consts ----
            bq_sb = cpool.tile([128, 3], FP)
            nc.scalar.dma_start(out=bq_sb[:], in_=bqkv[:, :])
            onesr_sb = cpool.tile([33, 128], FP)
            nc.scalar.dma_start(out=onesr_sb[:], in_=onesr[:, :])
            wout_sb = cpool.tile([128, C], BF)
            nc.sync.dma_start(out=wout_sb[:], in_=wout[:, :])

            # two static rcsum staging tiles (alternated per chunk), zeroed
            # once so selector rows 1-31 always multiply zeros
            rc_tiles = []
            for i in range(2):
                rct = cpool.tile([33, TC], FP, name=f"rcst{i}")
                nc.vector.memset(rct[:], 0.0)
                rc_tiles.append(rct)
            ident = cpool.tile([128, 128], BF)
            make_identity(nc, ident)

            # ---- multiplicative causal masks, generated on gpsimd ----
            # masks_sb[p, v*1024 + h*512 + q] = 1.0 if p + 128v <= q else 0
            masks_sb = cpool.tile([128, 4 * 1024], BF)
            nc.gpsimd.memset(masks_sb[:], 1.0)
            for v in range(4):
                nc.gpsimd.affine_select(
                    out=masks_sb[:, v * 1024:(v + 1) * 1024].rearrange(
                        "p (h q) -> p h q", h=2),
                    in_=masks_sb[:, v * 1024:(v + 1) * 1024].rearrange(
                        "p (h q) -> p h q", h=2),
                    compare_op=mybir.AluOpType.is_ge,
                    fill=0.0,
                    base=-128 * v,
                    pattern=[[0, 2], [1, TC]],
                    channel_multiplier=-1,
                )

            # ---- persistent intermediates ----
            QT = bigpool.tile([128, BT], BF)
            KT = bigpool.tile([128, BT], BF)
            # V in [token, dim] layout, 130 cols per 128-token block:
            # [V_h0 (64) | ones | V_h1 (64) | ones]
            vaug = bigpool.tile([128, 32 * 130], BF)
            nc.gpsimd.memset(
                vaug[:].rearrange("p (j a c) -> p j a c", a=2, c=65)[
                    :, :, :, 64:65], 1.0,
            )

            def phase_a(pair):
                """QKV projection + V transpose for a PAIR of adjacent
                512-token chunks (1024 tokens)."""
                t0 = pair * 2 * TC
                W = 2 * TC
                xtile = xtiles[pair]
                vt = None
                for g in range(3):
                    # share the "s" PSUM ring with phase B (8-bank budget);
                    # chunk c0 in bank 0, c1 in bank 1
                    ps = pspool.tile([128, 2 * TC], FP, tag="s", bufs=2)
                    for k in range(8):
                        for c in range(2):
                            nc.tensor.matmul(
                                ps[:, c * TC:(c + 1) * TC],
                                wc_sb[:, k * 384 + g * 128:
                                      k * 384 + (g + 1) * 128],
                                xtile[:, k * W + c * TC:k * W + (c + 1) * TC],
                                start=(k == 0),
                                stop=(k == 7),
                                skip_group_check=True,
                            )
                    if g < 2:
                        dst = (QT, KT)[g]
                        nc.scalar.activation(
                            dst[:, t0:t0 + W], ps[:], ACT.Identity,
                            bias=bq_sb[:, g:g + 1],
                        )
                    else:
                        vt = sbpool.tile([128, W], BF, tag="vt", bufs=2)
                        nc.scalar.activation(
                            vt[:], ps[:], ACT.Identity, bias=bq_sb[:, 2:3],
                        )
                # V transpose into vaug: PE transpose per 128-token block
                # (bf16, into a y-ring bank bitcast to bf16), then one
                # strided DVE copy splits the two heads around the ones col
                for j in range(8):
                    jj = pair * 8 + j
                    tp = pspool.tile([128, TC], FP, tag="y", bufs=2,
                                     name="tp")
                    tpb = tp[:].bitcast(BF)
                    nc.tensor.transpose(
                        tpb[:, 0:128], vt[:, j * 128:(j + 1) * 128], ident[:]
                    )
                    nc.vector.tensor_copy(
                        vaug[:].rearrange("p (j a c) -> p j a c", a=2, c=65)[
                            :, jj, :, 0:64],
                        tpb[:, 0:128].rearrange("p (a c) -> p a c", c=64),
                    )

            def attn(tcx, stages=None):
                """Scores/exp/AV for one (batch, q-chunk). Returns a list of
                finish-stage closures (normalize + out-projection) that the
                NEXT chunk emits at k-tiles 0..3, so the PE never sits
                behind the DVE normalize chain, and the av PSUM ring
                (bufs=1) frees before this chunk's first AV needs it."""
                b, qc = divmod(tcx, 4)
                t0 = tcx * TC
                n_kt = 4 * (qc + 1)
                stages = list(stages or [])

                otp = pspool.tile([65, 2 * TC], FP, tag="av", bufs=1,
                                  name="otp")
                pts = {}

                def emit_av(j):
                    kg = b * 16 + j
                    pt = pts.pop(j)
                    for h in range(2):
                        nc.tensor.matmul(
                            otp[:, h * TC:(h + 1) * TC],
                            vaug[:, kg * 130 + h * 65:kg * 130 + h * 65 + 65],
                            pt[:, h * TC:(h + 1) * TC],
                            start=(j == 0), stop=(j == n_kt - 1),
                            skip_group_check=True,
                        )

                for kt in range(n_kt):
                    kg = b * 16 + kt
                    sp = pspool.tile([128, 2 * TC], FP, tag="s", bufs=2)
                    for h in range(2):
                        nc.tensor.matmul(
                            sp[:, h * TC:(h + 1) * TC],
                            KT[h * 64:(h + 1) * 64, kg * 128:(kg + 1) * 128],
                            QT[h * 64:(h + 1) * 64, t0:t0 + TC],
                            start=True, stop=True,
                        )
                    pt = sbpool.tile([128, 2 * TC], BF, tag="pt",
                                     bufs=AV_DELAY + 4)
                    nc.scalar.activation(pt[:], sp[:], ACT.Exp, scale=0.125)
                    if kt >= 4 * qc:
                        v = kt - 4 * qc
                        nc.vector.tensor_mul(
                            pt[:], pt[:],
                            masks_sb[:, v * 1024:(v + 1) * 1024],
                        )
                    pts[kt] = pt
                    if kt < len(stages):
                        stages[kt]()
                    if kt >= AV_DELAY:
                        emit_av(kt - AV_DELAY)
                for j in range(max(n_kt - AV_DELAY, 0), n_kt):
                    emit_av(j)

                # finish stages; state shared via closure cells
                bc_cell, bcs_cell, ots_cell = [None], [None], [None]

                def st0():
                    rcsum = rc_tiles[tcx % 2]
                    nc.vector.tensor_copy(rcsum[0:1, :], otp[64:65, 0:TC])
                    nc.vector.tensor_copy(rcsum[32:33, :],
                                          otp[64:65, TC:2 * TC])
                    bc = pspool.tile([128, TC], FP, tag="y", bufs=2,
                                     name="bc")
                    nc.tensor.matmul(bc[:], onesr_sb[:], rc_tiles[tcx % 2][:],
                                     start=True, stop=True)
                    bc_cell[0] = bc

                def st1():
                    bcs = sbpool.tile([128, TC], FP, tag="bcs", bufs=2,
                                      name="bcs")
                    nc.vector.reciprocal_approx_fast(out=bcs[:],
                                                     in_=bc_cell[0][:])
                    bcs_cell[0] = bcs

                def st2():
                    bcs = bcs_cell[0]
                    ots = sbpool.tile([128, TC], BF, tag="ot", bufs=2,
                                      name="ot")
                    nc.vector.tensor_mul(ots[0:64, :], otp[0:64, 0:TC],
                                         bcs[0:64, :])
                    nc.vector.tensor_mul(ots[64:128, :], otp[0:64, TC:2 * TC],
                                         bcs[64:128, :])
                    ots_cell[0] = ots

                def st3():
                    # ---- out-projection: full 128-contract per m-tile ----
                    ots = ots_cell[0]
                    ys = sbpool.tile([128, 8 * TC], BF, tag="ys", bufs=2)
                    for m in range(8):
                        yp = pspool.tile([128, TC], FP, tag="y", bufs=2)
                        nc.tensor.matmul(
                            yp[:], wout_sb[:, m * 128:(m + 1) * 128], ots[:],
                            start=True, stop=True,
                        )
                        if m % 2 == 0:
                            nc.scalar.copy(ys[:, m * TC:(m + 1) * TC], yp[:])
                        else:
                            nc.vector.tensor_copy(
                                ys[:, m * TC:(m + 1) * TC], yp[:])
                    eng = nc.sync if tcx % 2 == 0 else nc.scalar
                    eng.dma_start(
                        out=ypt[tcx * 128:(tcx + 1) * 128, :], in_=ys[:],
                    )

                return [st0, st1, st2, st3]

            # ---- schedule ----
            # batch-0 QKV pairs; then attention chunks with batch-1 QKV
            # pairs interleaved as PE filler. Each chunk's finish stages
            # (normalize + out-projection) are emitted at k-tiles 0-3 of
            # the NEXT chunk: before that chunk's first AV (the av ring
            # bufs=1 would deadlock otherwise) and spread out so the PE
            # never waits on the DVE normalize chain.
            phase_a(0)
            phase_a(1)
            st = attn(0)
            st = attn(1, st)
            phase_a(2)
            st = attn(2, st)
            st = attn(3, st)
            phase_a(3)
            for qc in range(4):
                st = attn(4 + qc, st)
            for s in st:
                s()

    nc.compile()
    return nc


def make_in_maps(x, w_qkv, b_qkv):
    x = np.ascontiguousarray(np.asarray(x, np.float32).reshape(BT, C))
    xT = x.T.astype(ml_dtypes.bfloat16)          # [C, BT]
    # pack to [pair*128 + p, k*1024 + q]: xp[pr*128+p, k*1024+q]
    # = xT[k*128+p, pr*1024+q]
    xpack = np.ascontiguousarray(
        xT.reshape(8, 128, 4, 1024).transpose(2, 1, 0, 3).reshape(512, 8192)
    )
    w_qkv = np.asarray(w_qkv, np.float32)
    b_qkv = np.asarray(b_qkv, np.float32)

    onesr = np.zeros((33, 128), np.float32)
    onesr[0, 0:64] = 1.0
    onesr[32, 64:128] = 1.0

    in_maps = []
    for c in range(NCORES):
        sl = slice(c * 128, (c + 1) * 128)
        wcs = np.stack(
            [w_qkv[:, sl], w_qkv[:, 1024:][:, sl], w_qkv[:, 2048:][:, sl]],
            axis=1,
        )  # [1024, 3, 128]
        # wcp[p, k*384 + g*128 + cc] = wcs[k*128 + p, g, cc]
        wcp = np.ascontiguousarray(
            wcs.reshape(8, 128, 3, 128).transpose(1, 0, 2, 3).reshape(128, 3072)
        )
        bq = np.stack(
            [b_qkv[sl], b_qkv[1024:][sl], b_qkv[2048:][sl]], axis=1
        )
        in_maps.append({
            "xp": xpack,
            "wcp": wcp.astype(ml_dtypes.bfloat16),
            "wout": None,  # filled by caller (needs w_out)
            "bqkv": np.ascontiguousarray(bq),
            "onesr": onesr,
        })
    return in_maps


_NC_CACHE = None


def kernel(x, w_qkv, b_qkv, w_out, b_out):
    global _NC_CACHE, LAST_RESULTS
    if _NC_CACHE is None:
        _NC_CACHE = build_nc()
    nc = _NC_CACHE

    w_out = np.asarray(w_out, np.float32)
    in_maps = make_in_maps(x, w_qkv, b_qkv)
    for c in range(NCORES):
        in_maps[c]["wout"] = np.ascontiguousarray(
            w_out[c * 128:(c + 1) * 128, :]).astype(ml_dtypes.bfloat16)

    res = run_bass_kernel_spmd(
        nc, in_maps, list(range(NCORES)),
        trace=bool(os.environ.get("BASS_TRACE")),
    )
    LAST_RESULTS = res

    acc = np.zeros((C, BT), np.float32)
    for out_map in res.results:
        # ypt[tcx*128 + p, m*512 + q] -> y[m*128 + p, tcx*512 + q]
        yp = out_map["ypt"].astype(np.float32)
        acc += yp.reshape(8, 128, 8, 512).transpose(2, 1, 0, 3).reshape(
            C, BT)
    y = acc.T + np.asarray(b_out, np.float32)[None, :]
    return y.reshape(B, T, C)


# revision 26
# speedup vs baseline: 1.0207x; 1.0207x over previous
"""Multi-head causal attention (B=2, T=2048, C=1024, H=16) on 8 Trainium2
NeuronCores, tensor-parallel over heads (2 heads per core).

v5 — unit-scheduled software pipeline on top of the v4 data layout:
  - Host pre-packs x, w_qkv, and the y output into exact SBUF layouts so
    DMAs move 8-16KB contiguous per partition; x pair-loads are issued up
    front across both HWDGE queues (sync + scalar), the first pair in
    quarters so the PE starts ~5us earlier.
  - Causal masks + vaug ones-columns generated on-device (gpsimd).
  - Phase A (QKV) runs on chunk pairs sharing the "s" PSUM ring.
  - Phase B: both heads' scores in one 2-bank PSUM tile, ONE exp per
    k-tile, multiplicative bf16 mask after exp on diagonal tiles, AV with
    the ones-column denominator trick, stacked normalize (selector
    matmul + fast reciprocal + cross-partition muls), 128-contract
    out-projection, packed bf16 y DMAs split per m-pair across queues.
  - Unit scheduler: each chunk's TAIL work (last AVs, normalize stages,
    out-projection) is emitted inside the next chunk's early k-tiles or
    inside the interleaved batch-1 QKV pair, so the scalar engine's exp
    stream never starves at chunk boundaries and the av PSUM ring
    (bufs=1) frees before the next chunk's first AV.

Matmuls in bf16 (fp32 PSUM); softmax normalization in fp32.
"""

import os
import sys

for _p in ("/opt/trn_rl_repo", "/root/.axon_site/_ro/trn_rl_repo"):
    if os.path.isdir(_p) and _p not in sys.path:
        sys.path.insert(0, _p)

import ml_dtypes
import numpy as np

import concourse.bacc as bacc
import concourse.bass as bass
import concourse.mybir as mybir
import concourse.tile as tile
from concourse.bass_utils import run_bass_kernel_spmd
from concourse.masks import make_identity

B, T, C, H, D = 2, 2048, 1024, 16, 64
NCORES = 8
BT = B * T                      # 4096 flattened tokens
TC = 512                        # token chunk (matmul free dim)
NTC = BT // TC                  # 8 token chunks
FP = mybir.dt.float32
FPR = mybir.dt.float32r
BF = mybir.dt.bfloat16
ACT = mybir.ActivationFunctionType
AV_DELAY = 3                    # k-tiles the AV matmul trails the scores

LAST_RESULTS = None             # stashed BassKernelResults for test harness


def build_nc():
    nc = bacc.Bacc(None, target_bir_lowering=False, debug=False)

    # xp[pair*128 + p, k*1024 + q] = x[token, cin] with cin = k*128 + p,
    # token = pair*1024 + q  (SBUF layout, contiguous per partition)
    xp = nc.declare_dram_parameter("xp", [512, 8 * 1024], BF, isOutput=False)
    # wcp[p, k*384 + g*128 + c] = w_qkv[k*128 + p, g*1024 + core*128 + c]
    wcp = nc.declare_dram_parameter("wcp", [128, 3072], BF, isOutput=False)
    wout = nc.declare_dram_parameter("wout", [128, C], BF, isOutput=False)
    bqkv = nc.declare_dram_parameter("bqkv", [128, 3], FP, isOutput=False)
    # [33, 128] selector: row 0 -> output partitions 0-63 (head 0), row 32
    # -> partitions 64-127 (head 1), rows 1-31 zero (engine partition
    # bases must be multiples of 32)
    onesr = nc.declare_dram_parameter("onesr", [33, 128], FP, isOutput=False)
    # ypt[tcx*128 + p, m*512 + q] = y_partial[m*128 + p, tcx*512 + q]
    ypt = nc.declare_dram_parameter("ypt", [NTC * 128, 8 * TC], BF,
                                    isOutput=True)

    with tile.TileContext(nc) as tc:
        with (
            tc.tile_pool(name="const", bufs=1) as cpool,
            tc.tile_pool(name="big", bufs=1) as bigpool,
            tc.tile_pool(name="sb", bufs=2) as sbpool,
            tc.tile_pool(name="ps", bufs=2, space="PSUM") as pspool,
        ):
            # ---- weights + x prefetch, split across both HWDGE queues ----
            wc_sb = cpool.tile([128, 3072], BF)
            nc.sync.dma_start(out=wc_sb[:, 0:1536], in_=wcp[:, 0:1536])
            nc.scalar.dma_start(out=wc_sb[:, 1536:3072],
                                in_=wcp[:, 1536:3072])
            xtiles = []
            for pair in range(4):
                xtile = sbpool.tile([128, 8 * 1024], BF, tag="xt", bufs=4)
                src = xp[pair * 128:(pair + 1) * 128, :]
                if pair == 0:   # quarters so the PE can start sooner
                    nc.sync.dma_start(out=xtile[:, 0:2048],
                                      in_=src[:, 0:2048])
                    nc.scalar.dma_start(out=xtile[:, 2048:4096],
                                        in_=src[:, 2048:4096])
                    nc.sync.dma_start(out=xtile[:, 4096:6144],
                                      in_=src[:, 4096:6144])
                    nc.scalar.dma_start(out=xtile[:, 6144:8192],
                                        in_=src[:, 6144:8192])
                else:
                    nc.sync.dma_start(out=xtile[:, 0:4096],
                                      in_=src[:, 0:4096])
                    nc.scalar.dma_start(out=xtile[:, 4096:8192],
                                        in_=src[:, 4096:8192])
                xtiles.append(xtile)

            # ---- small consts ----
            bq_sb = cpool.tile([128, 3], FP)
            nc.scalar.dma_start(out=bq_sb[:], in_=bqkv[:, :])
            onesr_sb = cpool.tile([33, 128], FP)
            nc.scalar.dma_start(out=onesr_sb[:], in_=onesr[:, :])
            wout_sb = cpool.tile([128, C], BF)
            nc.sync.dma_start(out=wout_sb[:], in_=wout[:, :])

            # two static rcsum staging tiles (alternated per chunk), zeroed
            # once so selector rows 1-31 always multiply zeros
            rc_tiles = []
            for i in range(2):
                rct = cpool.tile([33, TC], FP, name=f"rcst{i}")
                nc.vector.memset(rct[:], 0.0)
                rc_tiles.append(rct)
            ident = cpool.tile([128, 128], BF)
            make_identity(nc, ident)

            # ---- multiplicative causal masks, generated on gpsimd ----
            # masks_sb[p, v*1024 + h*512 + q] = 1.0 if p + 128v <= q else 0
            masks_sb = cpool.tile([128, 4 * 1024], BF)
            nc.gpsimd.memset(masks_sb[:], 1.0)
            for v in range(4):
                nc.gpsimd.affine_select(
                    out=masks_sb[:, v * 1024:(v + 1) * 1024].rearrange(
                        "p (h q) -> p h q", h=2),
                    in_=masks_sb[:, v * 1024:(v + 1) * 1024].rearrange(
                        "p (h q) -> p h q", h=2),
                    compare_op=mybir.AluOpType.is_ge,
                    fill=0.0,
                    base=-128 * v,
                    pattern=[[0, 2], [1, TC]],
                    channel_multiplier=-1,
                )

            # ---- persistent intermediates ----
            QT = bigpool.tile([128, BT], BF)
            KT = bigpool.tile([128, BT], BF)
            # V in [token, dim] layout, 130 cols per 128-token block:
            # [V_h0 (64) | ones | V_h1 (64) | ones]
            vaug = bigpool.tile([128, 32 * 130], BF)
            nc.gpsimd.memset(
                vaug[:].rearrange("p (j a c) -> p j a c", a=2, c=65)[
                    :, :, :, 64:65], 1.0,
            )

            def phase_a(pair, tail=()):
                """QKV projection + V transpose for a PAIR of adjacent
                512-token chunks; `tail` units (the previous attention
                chunk's last AVs + finish stages) are interleaved between
                the g-groups so the scalar engine keeps busy."""
                t0 = pair * 2 * TC
                W = 2 * TC
                xtile = xtiles[pair]
                tail = list(tail)
                vt = None
                for g in range(3):
                    ps = pspool.tile([128, 2 * TC], FP, tag="s", bufs=2)
                    for k in range(8):
                        for c in range(2):
                            nc.tensor.matmul(
                                ps[:, c * TC:(c + 1) * TC],
                                wc_sb[:, k * 384 + g * 128:
                                      k * 384 + (g + 1) * 128],
                                xtile[:, k * W + c * TC:k * W + (c + 1) * TC],
                                start=(k == 0),
                                stop=(k == 7),
                                skip_group_check=True,
                            )
                    if g < 2:
                        dst = (QT, KT)[g]
                        nc.scalar.activation(
                            dst[:, t0:t0 + W], ps[:], ACT.Identity,
                            bias=bq_sb[:, g:g + 1],
                        )
                    else:
                        vt = sbpool.tile([128, W], BF, tag="vt", bufs=2)
                        nc.scalar.activation(
                            vt[:], ps[:], ACT.Identity, bias=bq_sb[:, 2:3],
                        )
                    for u in tail[g * 3:(g + 1) * 3]:
                        u()
                for u in tail[9:]:
                    u()
                # V transpose into vaug: PE transpose per 128-token block
                # (bf16, into a y-ring bank bitcast to bf16), then one
                # strided DVE copy splits the two heads around the ones col
                for j in range(8):
                    jj = pair * 8 + j
                    tp = pspool.tile([128, TC], FP, tag="y", bufs=2,
                                     name="tp")
                    tpb = tp[:].bitcast(BF)
                    nc.tensor.transpose(
                        tpb[:, 0:128], vt[:, j * 128:(j + 1) * 128], ident[:]
                    )
                    nc.vector.tensor_copy(
                        vaug[:].rearrange("p (j a c) -> p j a c", a=2, c=65)[
                            :, jj, :, 0:64],
                        tpb[:, 0:128].rearrange("p (a c) -> p a c", c=64),
                    )

            def attn(tcx, tail=()):
                """Scores/exp/AV for one (batch, q-chunk); the PREVIOUS
                chunk's tail units are interleaved into the first k-tiles
                (all before this chunk's first AV at k-tile AV_DELAY, so
                the av PSUM ring bufs=1 cannot deadlock). Returns this
                chunk's own tail units: [last AVs..., normalize stages,
                out-projection]."""
                b, qc = divmod(tcx, 4)
                t0 = tcx * TC
                n_kt = 4 * (qc + 1)
                tail = list(tail)

                otp = pspool.tile([65, 2 * TC], FP, tag="av", bufs=1,
                                  name="otp")
                pts = {}

                def emit_av(j):
                    kg = b * 16 + j
                    pt = pts.pop(j)
                    for h in range(2):
                        nc.tensor.matmul(
                            otp[:, h * TC:(h + 1) * TC],
                            vaug[:, kg * 130 + h * 65:kg * 130 + h * 65 + 65],
                            pt[:, h * TC:(h + 1) * TC],
                            start=(j == 0), stop=(j == n_kt - 1),
                            skip_group_check=True,
                        )

                # previous-chunk tail spread: slots at k-tiles 0,1,2 get
                # ceil(len/3) units each; everything lands before kt3
                per = (len(tail) + 2) // 3 if tail else 0

                for kt in range(n_kt):
                    kg = b * 16 + kt
                    sp = pspool.tile([128, 2 * TC], FP, tag="s", bufs=2)
                    for h in range(2):
                        nc.tensor.matmul(
                            sp[:, h * TC:(h + 1) * TC],
                            KT[h * 64:(h + 1) * 64, kg * 128:(kg + 1) * 128],
                            QT[h * 64:(h + 1) * 64, t0:t0 + TC],
                            start=True, stop=True,
                        )
                    pt = sbpool.tile([128, 2 * TC], BF, tag="pt",
                                     bufs=AV_DELAY + 4)
                    nc.scalar.activation(pt[:], sp[:], ACT.Exp, scale=0.125)
                    if kt >= 4 * qc:
                        v = kt - 4 * qc
                        nc.vector.tensor_mul(
                            pt[:], pt[:],
                            masks_sb[:, v * 1024:(v + 1) * 1024],
                        )
                    pts[kt] = pt
                    if kt < 3:
                        for u in tail[kt * per:(kt + 1) * per]:
                            u()
                    if kt >= AV_DELAY:
                        emit_av(kt - AV_DELAY)

                my_tail = []
                for j in range(max(n_kt - AV_DELAY, 0), n_kt):
                    my_tail.append(lambda j=j: emit_av(j))

                bc_cell, bcs_cell, ots_cell = [None], [None], [None]

                def st0():
                    rcsum = rc_tiles[tcx % 2]
                    nc.vector.tensor_copy(rcsum[0:1, :], otp[64:65, 0:TC])
                    nc.vector.tensor_copy(rcsum[32:33, :],
                                          otp[64:65, TC:2 * TC])
                    bc = pspool.tile([128, TC], FP, tag="y", bufs=2,
                                     name="bc")
                    nc.tensor.matmul(bc[:], onesr_sb[:], rc_tiles[tcx % 2][:],
                                     start=True, stop=True)
                    bc_cell[0] = bc

                def st1():
                    bcs = sbpool.tile([128, TC], FP, tag="bcs", bufs=2,
                                      name="bcs")
                    nc.vector.reciprocal_approx_fast(out=bcs[:],
                                                     in_=bc_cell[0][:])
                    bcs_cell[0] = bcs

                def st2():
                    bcs = bcs_cell[0]
                    ots = sbpool.tile([128, TC], BF, tag="ot", bufs=2,
                                      name="ot")
                    nc.vector.tensor_mul(ots[0:64, :], otp[0:64, 0:TC],
                                         bcs[0:64, :])
                    nc.vector.tensor_mul(ots[64:128, :], otp[0:64, TC:2 * TC],
                                         bcs[64:128, :])
                    ots_cell[0] = ots

                ys_cell = [None]

                def proj(half):
                    # out-projection (full 128-contract per m-tile), y DMA
                    # per 2048-col half, queues alternated
                    ots = ots_cell[0]
                    if half == 0:
                        ys_cell[0] = sbpool.tile([128, 8 * TC], BF, tag="ys",
                                                 bufs=2, name="ys")
                    ys = ys_cell[0]
                    for m in range(4 * half, 4 * half + 4):
                        yp = pspool.tile([128, TC], FP, tag="y", bufs=2)
                        nc.tensor.matmul(
                            yp[:], wout_sb[:, m * 128:(m + 1) * 128], ots[:],
                            start=True, stop=True,
                        )
                        if m % 2 == 0:
                            nc.scalar.copy(ys[:, m * TC:(m + 1) * TC], yp[:])
                        else:
                            nc.vector.tensor_copy(
                                ys[:, m * TC:(m + 1) * TC], yp[:])
                    eng = nc.sync if (tcx + half) % 2 == 0 else nc.scalar
                    eng.dma_start(
                        out=ypt[tcx * 128:(tcx + 1) * 128,
                                half * 4 * TC:(half + 1) * 4 * TC],
                        in_=ys[:, half * 4 * TC:(half + 1) * 4 * TC],
                    )

                my_tail += [st0, st1, st2,
                            lambda: proj(0), lambda: proj(1)]
                return my_tail

            # ---- schedule ----
            # batch-0 QKV pairs; attention chunks with batch-1 QKV pairs
            # interleaved; every chunk's tail rides inside the next
            # chunk's early k-tiles (or inside the interleaved QKV pair).
            phase_a(0)
            phase_a(1)
            tail = attn(0)
            tail = attn(1, tail)
            phase_a(2, tail)
            tail = attn(2)
            tail = attn(3, tail)
            phase_a(3, tail)
            tail = attn(4)
            for qc in range(1, 4):
                tail = attn(4 + qc, tail)
            for u in tail:
                u()

    nc.compile()
    return nc


def make_in_maps(x, w_qkv, b_qkv):
    x = np.ascontiguousarray(np.asarray(x, np.float32).reshape(BT, C))
    xT = x.T.astype(ml_dtypes.bfloat16)          # [C, BT]
    # xp[pr*128+p, k*1024+q] = xT[k*128+p, pr*1024+q]
    xpack = np.ascontiguousarray(
        xT.reshape(8, 128, 4, 1024).transpose(2, 1, 0, 3).reshape(512, 8192)
    )
    w_qkv = np.asarray(w_qkv, np.float32)
    b_qkv = np.asarray(b_qkv, np.float32)

    onesr = np.zeros((33, 128), np.float32)
    onesr[0, 0:64] = 1.0
    onesr[32, 64:128] = 1.0

    in_maps = []
    for c in range(NCORES):
        sl = slice(c * 128, (c + 1) * 128)
        wcs = np.stack(
            [w_qkv[:, sl], w_qkv[:, 1024:][:, sl], w_qkv[:, 2048:][:, sl]],
            axis=1,
        )  # [1024, 3, 128]
        # wcp[p, k*384 + g*128 + cc] = wcs[k*128 + p, g, cc]
        wcp = np.ascontiguousarray(
            wcs.reshape(8, 128, 3, 128).transpose(1, 0, 2, 3).reshape(128, 3072)
        )
        bq = np.stack(
            [b_qkv[sl], b_qkv[1024:][sl], b_qkv[2048:][sl]], axis=1
        )
        in_maps.append({
            "xp": xpack,
            "wcp": wcp.astype(ml_dtypes.bfloat16),
            "wout": None,  # filled by caller (needs w_out)
            "bqkv": np.ascontiguousarray(bq),
            "onesr": onesr,
        })
    return in_maps


_NC_CACHE = None


def kernel(x, w_qkv, b_qkv, w_out, b_out):
    global _NC_CACHE, LAST_RESULTS
    if _NC_CACHE is None:
        _NC_CACHE = build_nc()
    nc = _NC_CACHE

    w_out = np.asarray(w_out, np.float32)
    in_maps = make_in_maps(x, w_qkv, b_qkv)
    for c in range(NCORES):
        in_maps[c]["wout"] = np.ascontiguousarray(
            w_out[c * 128:(c + 1) * 128, :]).astype(ml_dtypes.bfloat16)

    res = run_bass_kernel_spmd(
        nc, in_maps, list(range(NCORES)),
        trace=bool(os.environ.get("BASS_TRACE")),
    )
    LAST_RESULTS = res

    acc = np.zeros((C, BT), np.float32)
    for out_map in res.results:
        # ypt[tcx*128 + p, m*512 + q] -> y[m*128 + p, tcx*512 + q]
        yp = out_map["ypt"].astype(np.float32)
        acc += yp.reshape(8, 128, 8, 512).transpose(2, 1, 0, 3).reshape(
            C, BT)
    y = acc.T + np.asarray(b_out, np.float32)[None, :]
    return y.reshape(B, T, C)


# revision 28
# speedup vs baseline: 1.0211x; 1.0004x over previous
"""Multi-head causal attention (B=2, T=2048, C=1024, H=16) on 8 Trainium2
NeuronCores, tensor-parallel over heads (2 heads per core).

v5 — unit-scheduled software pipeline on top of the v4 data layout:
  - Host pre-packs x, w_qkv, and the y output into exact SBUF layouts so
    DMAs move 8-16KB contiguous per partition; x pair-loads are issued up
    front across both HWDGE queues (sync + scalar), the first pair in
    quarters so the PE starts ~5us earlier.
  - Causal masks + vaug ones-columns generated on-device (gpsimd).
  - Phase A (QKV) runs on chunk pairs sharing the "s" PSUM ring.
  - Phase B: both heads' scores in one 2-bank PSUM tile, ONE exp per
    k-tile, multiplicative bf16 mask after exp on diagonal tiles, AV with
    the ones-column denominator trick, stacked normalize (selector
    matmul + fast reciprocal + cross-partition muls), 128-contract
    out-projection, packed bf16 y DMAs split per m-pair across queues.
  - Unit scheduler: each chunk's TAIL work (last AVs, normalize stages,
    out-projection) is emitted inside the next chunk's early k-tiles or
    inside the interleaved batch-1 QKV pair, so the scalar engine's exp
    stream never starves at chunk boundaries and the av PSUM ring
    (bufs=1) frees before the next chunk's first AV.

Matmuls in bf16 (fp32 PSUM); softmax normalization in fp32.
"""

import os
import sys

for _p in ("/opt/trn_rl_repo", "/root/.axon_site/_ro/trn_rl_repo"):
    if os.path.isdir(_p) and _p not in sys.path:
        sys.path.insert(0, _p)

import ml_dtypes
import numpy as np

import concourse.bacc as bacc
import concourse.bass as bass
import concourse.mybir as mybir
import concourse.tile as tile
from concourse.bass_utils import run_bass_kernel_spmd
from concourse.masks import make_identity

B, T, C, H, D = 2, 2048, 1024, 16, 64
NCORES = 8
BT = B * T                      # 4096 flattened tokens
TC = 512                        # token chunk (matmul free dim)
NTC = BT // TC                  # 8 token chunks
FP = mybir.dt.float32
FPR = mybir.dt.float32r
BF = mybir.dt.bfloat16
ACT = mybir.ActivationFunctionType
AV_DELAY = 3                    # k-tiles the AV matmul trails the scores

LAST_RESULTS = None             # stashed BassKernelResults for test harness


def build_nc():
    nc = bacc.Bacc(None, target_bir_lowering=False, debug=False)

    # xp[pair*128 + p, k*1024 + q] = x[token, cin] with cin = k*128 + p,
    # token = pair*1024 + q  (SBUF layout, contiguous per partition)
    xp = nc.declare_dram_parameter("xp", [512, 8 * 1024], BF, isOutput=False)
    # wcp[p, k*384 + g*128 + c] = w_qkv[k*128 + p, g*1024 + core*128 + c]
    wcp = nc.declare_dram_parameter("wcp", [128, 3072], BF, isOutput=False)
    wout = nc.declare_dram_parameter("wout", [128, C], BF, isOutput=False)
    bqkv = nc.declare_dram_parameter("bqkv", [128, 3], FP, isOutput=False)
    # [33, 128] selector: row 0 -> output partitions 0-63 (head 0), row 32
    # -> partitions 64-127 (head 1), rows 1-31 zero (engine partition
    # bases must be multiples of 32)
    onesr = nc.declare_dram_parameter("onesr", [33, 128], FP, isOutput=False)
    # ypt[tcx*128 + p, m*512 + q] = y_partial[m*128 + p, tcx*512 + q]
    ypt = nc.declare_dram_parameter("ypt", [NTC * 128, 8 * TC], BF,
                                    isOutput=True)

    with tile.TileContext(nc) as tc:
        with (
            tc.tile_pool(name="const", bufs=1) as cpool,
            tc.tile_pool(name="big", bufs=1) as bigpool,
            tc.tile_pool(name="sb", bufs=2) as sbpool,
            tc.tile_pool(name="ps", bufs=2, space="PSUM") as pspool,
        ):
            # ---- weights + x prefetch, split across both HWDGE queues ----
            wc_sb = cpool.tile([128, 3072], BF)
            nc.sync.dma_start(out=wc_sb[:, 0:1536], in_=wcp[:, 0:1536])
            nc.scalar.dma_start(out=wc_sb[:, 1536:3072],
                                in_=wcp[:, 1536:3072])
            xtiles = []
            for pair in range(4):
                xtile = sbpool.tile([128, 8 * 1024], BF, tag="xt", bufs=4)
                src = xp[pair * 128:(pair + 1) * 128, :]
                if pair == 0:   # quarters so the PE can start sooner
                    nc.sync.dma_start(out=xtile[:, 0:2048],
                                      in_=src[:, 0:2048])
                    nc.scalar.dma_start(out=xtile[:, 2048:4096],
                                        in_=src[:, 2048:4096])
                    nc.sync.dma_start(out=xtile[:, 4096:6144],
                                      in_=src[:, 4096:6144])
                    nc.scalar.dma_start(out=xtile[:, 6144:8192],
                                        in_=src[:, 6144:8192])
                else:
                    nc.sync.dma_start(out=xtile[:, 0:4096],
                                      in_=src[:, 0:4096])
                    nc.scalar.dma_start(out=xtile[:, 4096:8192],
                                        in_=src[:, 4096:8192])
                xtiles.append(xtile)

            # ---- small consts ----
            bq_sb = cpool.tile([128, 3], FP)
            nc.scalar.dma_start(out=bq_sb[:], in_=bqkv[:, :])
            onesr_sb = cpool.tile([33, 128], FP)
            nc.scalar.dma_start(out=onesr_sb[:], in_=onesr[:, :])
            wout_sb = cpool.tile([128, C], BF)
            nc.sync.dma_start(out=wout_sb[:], in_=wout[:, :])

            # two static rcsum staging tiles (alternated per chunk), zeroed
            # once so selector rows 1-31 always multiply zeros
            rc_tiles = []
            for i in range(2):
                rct = cpool.tile([33, TC], FP, name=f"rcst{i}")
                nc.vector.memset(rct[:], 0.0)
                rc_tiles.append(rct)
            ident = cpool.tile([128, 128], BF)
            make_identity(nc, ident)

            # ---- multiplicative causal masks, generated on gpsimd ----
            # masks_sb[p, v*1024 + h*512 + q] = 1.0 if p + 128v <= q else 0
            masks_sb = cpool.tile([128, 4 * 1024], BF)
            nc.gpsimd.memset(masks_sb[:], 1.0)
            for v in range(4):
                nc.gpsimd.affine_select(
                    out=masks_sb[:, v * 1024:(v + 1) * 1024].rearrange(
                        "p (h q) -> p h q", h=2),
                    in_=masks_sb[:, v * 1024:(v + 1) * 1024].rearrange(
                        "p (h q) -> p h q", h=2),
                    compare_op=mybir.AluOpType.is_ge,
                    fill=0.0,
                    base=-128 * v,
                    pattern=[[0, 2], [1, TC]],
                    channel_multiplier=-1,
                )

            # ---- persistent intermediates ----
            QT = bigpool.tile([128, BT], BF)
            KT = bigpool.tile([128, BT], BF)
            # V in [token, dim] layout, 130 cols per 128-token block:
            # [V_h0 (64) | ones | V_h1 (64) | ones]
            vaug = bigpool.tile([128, 32 * 130], BF)
            nc.gpsimd.memset(
                vaug[:].rearrange("p (j a c) -> p j a c", a=2, c=65)[
                    :, :, :, 64:65], 1.0,
            )

            def phase_a(pair, tail=()):
                """QKV projection + V transpose for a PAIR of adjacent
                512-token chunks; `tail` units (the previous attention
                chunk's last AVs + finish stages) are interleaved between
                the g-groups so the scalar engine keeps busy."""
                t0 = pair * 2 * TC
                W = 2 * TC
                xtile = xtiles[pair]
                tail = list(tail)
                vt = None
                for g in range(3):
                    ps = pspool.tile([128, 2 * TC], FP, tag="s", bufs=2)
                    for k in range(8):
                        for c in range(2):
                            nc.tensor.matmul(
                                ps[:, c * TC:(c + 1) * TC],
                                wc_sb[:, k * 384 + g * 128:
                                      k * 384 + (g + 1) * 128],
                                xtile[:, k * W + c * TC:k * W + (c + 1) * TC],
                                start=(k == 0),
                                stop=(k == 7),
                                skip_group_check=True,
                            )
                    if g < 2:
                        dst = (QT, KT)[g]
                        nc.scalar.activation(
                            dst[:, t0:t0 + W], ps[:], ACT.Identity,
                            bias=bq_sb[:, g:g + 1],
                        )
                    else:
                        vt = sbpool.tile([128, W], BF, tag="vt", bufs=2)
                        nc.scalar.activation(
                            vt[:], ps[:], ACT.Identity, bias=bq_sb[:, 2:3],
                        )
                    for u in tail[g * 3:(g + 1) * 3]:
                        u()
                for u in tail[9:]:
                    u()
                # V transpose into vaug: PE transpose per 128-token block
                # (bf16, into a y-ring bank bitcast to bf16), then one
                # strided DVE copy splits the two heads around the ones col
                for j in range(8):
                    jj = pair * 8 + j
                    tp = pspool.tile([128, TC], FP, tag="y", bufs=2,
                                     name="tp")
                    tpb = tp[:].bitcast(BF)
                    nc.tensor.transpose(
                        tpb[:, 0:128], vt[:, j * 128:(j + 1) * 128], ident[:]
                    )
                    nc.vector.tensor_copy(
                        vaug[:].rearrange("p (j a c) -> p j a c", a=2, c=65)[
                            :, jj, :, 0:64],
                        tpb[:, 0:128].rearrange("p (a c) -> p a c", c=64),
                    )

            def attn(tcx, tail=()):
                """Scores/exp/AV for one (batch, q-chunk); the PREVIOUS
                chunk's tail units are interleaved into the first k-tiles
                (all before this chunk's first AV at k-tile AV_DELAY, so
                the av PSUM ring bufs=1 cannot deadlock). Returns this
                chunk's own tail units: [last AVs..., normalize stages,
                out-projection]."""
                b, qc = divmod(tcx, 4)
                t0 = tcx * TC
                n_kt = 4 * (qc + 1)
                tail = list(tail)

                otp = pspool.tile([65, 2 * TC], FP, tag="av", bufs=1,
                                  name="otp")
                pts = {}

                def emit_av(j):
                    kg = b * 16 + j
                    pt = pts.pop(j)
                    for h in range(2):
                        nc.tensor.matmul(
                            otp[:, h * TC:(h + 1) * TC],
                            vaug[:, kg * 130 + h * 65:kg * 130 + h * 65 + 65],
                            pt[:, h * TC:(h + 1) * TC],
                            start=(j == 0), stop=(j == n_kt - 1),
                            skip_group_check=True,
                        )

                # previous-chunk tail spread: slots at k-tiles 0,1,2 get
                # ceil(len/3) units each; everything lands before kt3
                per = (len(tail) + 2) // 3 if tail else 0

                for kt in range(n_kt):
                    kg = b * 16 + kt
                    sp = pspool.tile([128, 2 * TC], FP, tag="s", bufs=2)
                    for h in range(2):
                        nc.tensor.matmul(
                            sp[:, h * TC:(h + 1) * TC],
                            KT[h * 64:(h + 1) * 64, kg * 128:(kg + 1) * 128],
                            QT[h * 64:(h + 1) * 64, t0:t0 + TC],
                            start=True, stop=True,
                        )
                    pt = sbpool.tile([128, 2 * TC], BF, tag="pt",
                                     bufs=AV_DELAY + 4)
                    nc.scalar.activation(pt[:], sp[:], ACT.Exp, scale=0.125)
                    if kt >= 4 * qc:
                        v = kt - 4 * qc
                        nc.vector.tensor_mul(
                            pt[:], pt[:],
                            masks_sb[:, v * 1024:(v + 1) * 1024],
                        )
                    pts[kt] = pt
                    if kt < 3:
                        for u in tail[kt * per:(kt + 1) * per]:
                            u()
                    if kt >= AV_DELAY:
                        emit_av(kt - AV_DELAY)

                my_tail = []
                for j in range(max(n_kt - AV_DELAY, 0), n_kt):
                    my_tail.append(lambda j=j: emit_av(j))

                bc_cell, bcs_cell, ots_cell = [None], [None], [None]

                def st0():
                    rcsum = rc_tiles[tcx % 2]
                    nc.vector.tensor_copy(rcsum[0:1, :], otp[64:65, 0:TC])
                    nc.vector.tensor_copy(rcsum[32:33, :],
                                          otp[64:65, TC:2 * TC])
                    bc = pspool.tile([128, TC], FP, tag="y", bufs=2,
                                     name="bc")
                    nc.tensor.matmul(bc[:], onesr_sb[:], rc_tiles[tcx % 2][:],
                                     start=True, stop=True)
                    bc_cell[0] = bc

                def st1():
                    bcs = sbpool.tile([128, TC], FP, tag="bcs", bufs=2,
                                      name="bcs")
                    nc.vector.reciprocal_approx_fast(out=bcs[:],
                                                     in_=bc_cell[0][:])
                    bcs_cell[0] = bcs

                def st2():
                    bcs = bcs_cell[0]
                    ots = sbpool.tile([128, TC], BF, tag="ot", bufs=2,
                                      name="ot")
                    nc.vector.tensor_mul(ots[0:64, :], otp[0:64, 0:TC],
                                         bcs[0:64, :])
                    nc.vector.tensor_mul(ots[64:128, :], otp[0:64, TC:2 * TC],
                                         bcs[64:128, :])
                    ots_cell[0] = ots

                ys_cell = [None]

                def proj(half):
                    # out-projection (full 128-contract per m-tile), y DMA
                    # per 2048-col half, queues alternated
                    ots = ots_cell[0]
                    if half == 0:
                        ys_cell[0] = sbpool.tile([128, 8 * TC], BF, tag="ys",
                                                 bufs=2, name="ys")
                    ys = ys_cell[0]
                    for m in range(4 * half, 4 * half + 4):
                        yp = pspool.tile([128, TC], FP, tag="y", bufs=2)
                        nc.tensor.matmul(
                            yp[:], wout_sb[:, m * 128:(m + 1) * 128], ots[:],
                            start=True, stop=True,
                        )
                        if m % 4 == 0:
                            nc.scalar.copy(ys[:, m * TC:(m + 1) * TC], yp[:])
                        else:
                            nc.vector.tensor_copy(
                                ys[:, m * TC:(m + 1) * TC], yp[:])
                    eng = nc.sync if (tcx + half) % 2 == 0 else nc.scalar
                    eng.dma_start(
                        out=ypt[tcx * 128:(tcx + 1) * 128,
                                half * 4 * TC:(half + 1) * 4 * TC],
                        in_=ys[:, half * 4 * TC:(half + 1) * 4 * TC],
                    )

                my_tail += [st0, st1, st2,
                            lambda: proj(0), lambda: proj(1)]
                return my_tail

            # ---- schedule ----
            # batch-0 QKV pairs with attn(0) between them (attn(0) only
            # needs pair 0, and its k-tiles cover pair 1's x-DMA
            # latency); batch-1 QKV pairs interleaved as PE filler; every
            # chunk's tail rides inside the next chunk's early k-tiles
            # (or inside the interleaved QKV pair).
            phase_a(0)
            tail = attn(0)
            phase_a(1, tail)
            tail = attn(1)
            phase_a(2, tail)
            tail = attn(2)
            tail = attn(3, tail)
            phase_a(3, tail)
            tail = attn(4)
            for qc in range(1, 4):
                tail = attn(4 + qc, tail)
            for u in tail:
                u()

    nc.compile()
    return nc


def make_in_maps(x, w_qkv, b_qkv):
    x = np.ascontiguousarray(np.asarray(x, np.float32).reshape(BT, C))
    xT = x.T.astype(ml_dtypes.bfloat16)          # [C, BT]
    # xp[pr*128+p, k*1024+q] = xT[k*128+p, pr*1024+q]
    xpack = np.ascontiguousarray(
        xT.reshape(8, 128, 4, 1024).transpose(2, 1, 0, 3).reshape(512, 8192)
    )
    w_qkv = np.asarray(w_qkv, np.float32)
    b_qkv = np.asarray(b_qkv, np.float32)

    onesr = np.zeros((33, 128), np.float32)
    onesr[0, 0:64] = 1.0
    onesr[32, 64:128] = 1.0

    in_maps = []
    for c in range(NCORES):
        sl = slice(c * 128, (c + 1) * 128)
        wcs = np.stack(
            [w_qkv[:, sl], w_qkv[:, 1024:][:, sl], w_qkv[:, 2048:][:, sl]],
            axis=1,
        )  # [1024, 3, 128]
        # wcp[p, k*384 + g*128 + cc] = wcs[k*128 + p, g, cc]
        wcp = np.ascontiguousarray(
            wcs.reshape(8, 128, 3, 128).transpose(1, 0, 2, 3).reshape(128, 3072)
        )
        bq = np.stack(
            [b_qkv[sl], b_qkv[1024:][sl], b_qkv[2048:][sl]], axis=1
        )
        in_maps.append({
            "xp": xpack,
            "wcp": wcp.astype(ml_dtypes.bfloat16),
            "wout": None,  # filled by caller (needs w_out)
            "bqkv": np.ascontiguousarray(bq),
            "onesr": onesr,
        })
    return in_maps


_NC_CACHE = None


def kernel(x, w_qkv, b_qkv, w_out, b_out):
    global _NC_CACHE, LAST_RESULTS
    if _NC_CACHE is None:
        _NC_CACHE = build_nc()
    nc = _NC_CACHE

    w_out = np.asarray(w_out, np.float32)
    in_maps = make_in_maps(x, w_qkv, b_qkv)
    for c in range(NCORES):
        in_maps[c]["wout"] = np.ascontiguousarray(
            w_out[c * 128:(c + 1) * 128, :]).astype(ml_dtypes.bfloat16)

    res = run_bass_kernel_spmd(
        nc, in_maps, list(range(NCORES)),
        trace=bool(os.environ.get("BASS_TRACE")),
    )
    LAST_RESULTS = res

    acc = np.zeros((C, BT), np.float32)
    for out_map in res.results:
        # ypt[tcx*128 + p, m*512 + q] -> y[m*128 + p, tcx*512 + q]
        yp = out_map["ypt"].astype(np.float32)
        acc += yp.reshape(8, 128, 8, 512).transpose(2, 1, 0, 3).reshape(
            C, BT)
    y = acc.T + np.asarray(b_out, np.float32)[None, :]
    return y.reshape(B, T, C)


# revision 37
# speedup vs baseline: 1.0493x; 1.0276x over previous
"""Multi-head causal attention (B=2, T=2048, C=1024, H=16) on 8 Trainium2
NeuronCores, tensor-parallel over heads (2 heads per core).

v5 — unit-scheduled software pipeline on top of the v4 data layout:
  - Host pre-packs x, w_qkv, and the y output into exact SBUF layouts so
    DMAs move 8-16KB contiguous per partition; x pair-loads are issued up
    front across both HWDGE queues (sync + scalar), the first pair in
    quarters so the PE starts ~5us earlier.
  - Causal masks + vaug ones-columns generated on-device (gpsimd).
  - Phase A (QKV) runs on chunk pairs sharing the "s" PSUM ring.
  - Phase B: both heads' scores in one 2-bank PSUM tile, ONE exp per
    k-tile, multiplicative bf16 mask after exp on diagonal tiles, AV with
    the ones-column denominator trick, stacked normalize (selector
    matmul + fast reciprocal + cross-partition muls), 128-contract
    out-projection, packed bf16 y DMAs split per m-pair across queues.
  - Unit scheduler: each chunk's TAIL work (last AVs, normalize stages,
    out-projection) is emitted inside the next chunk's early k-tiles or
    inside the interleaved batch-1 QKV pair, so the scalar engine's exp
    stream never starves at chunk boundaries and the av PSUM ring
    (bufs=1) frees before the next chunk's first AV.

Matmuls in bf16 (fp32 PSUM); softmax normalization in fp32.
"""

import os
import sys

for _p in ("/opt/trn_rl_repo", "/root/.axon_site/_ro/trn_rl_repo"):
    if os.path.isdir(_p) and _p not in sys.path:
        sys.path.insert(0, _p)

import ml_dtypes
import numpy as np

import concourse.bacc as bacc
import concourse.bass as bass
import concourse.mybir as mybir
import concourse.tile as tile
from concourse.bass_utils import run_bass_kernel_spmd
from concourse.masks import make_identity

B, T, C, H, D = 2, 2048, 1024, 16, 64
NCORES = 8
BT = B * T                      # 4096 flattened tokens
TC = 512                        # token chunk (matmul free dim)
NTC = BT // TC                  # 8 token chunks
FP = mybir.dt.float32
FPR = mybir.dt.float32r
BF = mybir.dt.bfloat16
ACT = mybir.ActivationFunctionType
AV_DELAY = 3                    # k-tiles the AV matmul trails the scores

LAST_RESULTS = None             # stashed BassKernelResults for test harness


def build_nc():
    nc = bacc.Bacc(None, target_bir_lowering=False, debug=False)

    # xp[pair*128 + p, k*1024 + q] = x[token, cin] with cin = k*128 + p,
    # token = pair*1024 + q  (SBUF layout, contiguous per partition)
    xp = nc.declare_dram_parameter("xp", [512, 8 * 1024], BF, isOutput=False)
    # wcp[p, k*384 + g*128 + c] = w_qkv[k*128 + p, g*1024 + core*128 + c]
    wcp = nc.declare_dram_parameter("wcp", [128, 3072], BF, isOutput=False)
    wout = nc.declare_dram_parameter("wout", [128, C], BF, isOutput=False)
    bqkv = nc.declare_dram_parameter("bqkv", [128, 3], FP, isOutput=False)
    # [33, 128] selector: row 0 -> output partitions 0-63 (head 0), row 32
    # -> partitions 64-127 (head 1), rows 1-31 zero (engine partition
    # bases must be multiples of 32)
    onesr = nc.declare_dram_parameter("onesr", [33, 128], FP, isOutput=False)
    # ypt[tcx*128 + p, m*512 + q] = y_partial[m*128 + p, tcx*512 + q]
    ypt = nc.declare_dram_parameter("ypt", [NTC * 128, 8 * TC], BF,
                                    isOutput=True)

    with tile.TileContext(nc) as tc:
        with (
            tc.tile_pool(name="const", bufs=1) as cpool,
            tc.tile_pool(name="big", bufs=1) as bigpool,
            tc.tile_pool(name="sb", bufs=2) as sbpool,
            tc.tile_pool(name="ps", bufs=2, space="PSUM") as pspool,
        ):
            # ---- weights + x prefetch, split across both HWDGE queues ----
            wc_sb = cpool.tile([128, 3072], BF)
            nc.sync.dma_start(out=wc_sb[:, 0:1536], in_=wcp[:, 0:1536])
            nc.scalar.dma_start(out=wc_sb[:, 1536:3072],
                                in_=wcp[:, 1536:3072])
            xtiles = []
            for pair in range(4):
                xtile = sbpool.tile([128, 8 * 1024], BF, tag="xt", bufs=4)
                src = xp[pair * 128:(pair + 1) * 128, :]
                if pair == 0:   # quarters so the PE can start sooner
                    nc.sync.dma_start(out=xtile[:, 0:2048],
                                      in_=src[:, 0:2048])
                    nc.scalar.dma_start(out=xtile[:, 2048:4096],
                                        in_=src[:, 2048:4096])
                    nc.sync.dma_start(out=xtile[:, 4096:6144],
                                      in_=src[:, 4096:6144])
                    nc.scalar.dma_start(out=xtile[:, 6144:8192],
                                        in_=src[:, 6144:8192])
                else:
                    nc.sync.dma_start(out=xtile[:, 0:4096],
                                      in_=src[:, 0:4096])
                    nc.scalar.dma_start(out=xtile[:, 4096:8192],
                                        in_=src[:, 4096:8192])
                xtiles.append(xtile)

            # ---- small consts ----
            bq_sb = cpool.tile([128, 3], FP)
            nc.scalar.dma_start(out=bq_sb[:], in_=bqkv[:, :])
            onesr_sb = cpool.tile([33, 128], FP)
            nc.scalar.dma_start(out=onesr_sb[:], in_=onesr[:, :])
            wout_sb = cpool.tile([128, C], BF)
            nc.sync.dma_start(out=wout_sb[:], in_=wout[:, :])

            # two static rcsum staging tiles (alternated per chunk), zeroed
            # once so selector rows 1-31 always multiply zeros
            rc_tiles = []
            for i in range(2):
                rct = cpool.tile([33, TC], FP, name=f"rcst{i}")
                nc.vector.memset(rct[:], 0.0)
                rc_tiles.append(rct)
            ident = cpool.tile([128, 128], BF)
            make_identity(nc, ident)

            # ---- multiplicative causal masks, generated on gpsimd ----
            # masks_sb[p, v*1024 + h*512 + q] = 1.0 if p + 128v <= q else 0
            masks_sb = cpool.tile([128, 4 * 1024], BF)
            nc.gpsimd.memset(masks_sb[:], 1.0)
            for v in range(4):
                nc.gpsimd.affine_select(
                    out=masks_sb[:, v * 1024:(v + 1) * 1024].rearrange(
                        "p (h q) -> p h q", h=2),
                    in_=masks_sb[:, v * 1024:(v + 1) * 1024].rearrange(
                        "p (h q) -> p h q", h=2),
                    compare_op=mybir.AluOpType.is_ge,
                    fill=0.0,
                    base=-128 * v,
                    pattern=[[0, 2], [1, TC]],
                    channel_multiplier=-1,
                )

            # ---- persistent intermediates ----
            QT = bigpool.tile([128, BT], BF)
            KT = bigpool.tile([128, BT], BF)
            # V in [token, dim] layout, 130 cols per 128-token block:
            # [V_h0 (64) | ones | V_h1 (64) | ones]
            vaug = bigpool.tile([128, 32 * 130], BF)
            nc.gpsimd.memset(
                vaug[:].rearrange("p (j a c) -> p j a c", a=2, c=65)[
                    :, :, :, 64:65], 1.0,
            )

            def phase_a(pair, tail=()):
                """QKV projection + V transpose for a PAIR of adjacent
                512-token chunks; `tail` units (the previous attention
                chunk's last AVs + finish stages) are interleaved between
                the g-groups so the scalar engine keeps busy."""
                t0 = pair * 2 * TC
                W = 2 * TC
                xtile = xtiles[pair]
                tail = list(tail)
                vt = None
                for g in range(3):
                    ps = pspool.tile([128, 2 * TC], FP, tag="s", bufs=2)
                    for k in range(8):
                        for c in range(2):
                            nc.tensor.matmul(
                                ps[:, c * TC:(c + 1) * TC],
                                wc_sb[:, k * 384 + g * 128:
                                      k * 384 + (g + 1) * 128],
                                xtile[:, k * W + c * TC:k * W + (c + 1) * TC],
                                start=(k == 0),
                                stop=(k == 7),
                                skip_group_check=True,
                            )
                    if g < 2:
                        dst = (QT, KT)[g]
                        nc.scalar.activation(
                            dst[:, t0:t0 + W], ps[:], ACT.Identity,
                            bias=bq_sb[:, g:g + 1],
                        )
                    else:
                        vt = sbpool.tile([128, W], BF, tag="vt", bufs=2)
                        nc.scalar.activation(
                            vt[:], ps[:], ACT.Identity, bias=bq_sb[:, 2:3],
                        )
                    for u in tail[g * 3:(g + 1) * 3]:
                        u()
                for u in tail[9:]:
                    u()
                # V transpose into vaug: PE transpose per 128-token block
                # (bf16, into a y-ring bank bitcast to bf16), then one
                # strided DVE copy splits the two heads around the ones col
                for j in range(8):
                    jj = pair * 8 + j
                    tp = pspool.tile([128, TC], FP, tag="y", bufs=2,
                                     name="tp")
                    tpb = tp[:].bitcast(BF)
                    nc.tensor.transpose(
                        tpb[:, 0:128], vt[:, j * 128:(j + 1) * 128], ident[:]
                    )
                    nc.vector.tensor_copy(
                        vaug[:].rearrange("p (j a c) -> p j a c", a=2, c=65)[
                            :, jj, :, 0:64],
                        tpb[:, 0:128].rearrange("p (a c) -> p a c", c=64),
                    )

            def attn(tcx, tail=()):
                """Scores/exp/AV for one (batch, q-chunk); the PREVIOUS
                chunk's tail units are interleaved into the first k-tiles
                (all before this chunk's first AV at k-tile AV_DELAY, so
                the av PSUM ring bufs=1 cannot deadlock). Returns this
                chunk's own tail units: [last AVs..., normalize stages,
                out-projection]."""
                b, qc = divmod(tcx, 4)
                t0 = tcx * TC
                n_kt = 4 * (qc + 1)
                tail = list(tail)

                otp = pspool.tile([65, 2 * TC], FP, tag="av", bufs=1,
                                  name="otp")
                pts = {}

                def emit_av(j):
                    kg = b * 16 + j
                    pt = pts.pop(j)
                    for h in range(2):
                        nc.tensor.matmul(
                            otp[:, h * TC:(h + 1) * TC],
                            vaug[:, kg * 130 + h * 65:kg * 130 + h * 65 + 65],
                            pt[:, h * TC:(h + 1) * TC],
                            start=(j == 0), stop=(j == n_kt - 1),
                            skip_group_check=True,
                        )

                # previous-chunk tail spread: the first 6 units (its last
                # AVs + normalize, which free the av ring) go 2-per-slot
                # at k-tiles 0-2; its out-projection halves ride at k-tiles
                # 3-4; leftovers (short chunks) emit right after the loop
                slots = {0: tail[0:2], 1: tail[2:4], 2: tail[4:6],
                         3: tail[6:7], 4: tail[7:8]}
                emitted = [0]

                for kt in range(n_kt):
                    kg = b * 16 + kt
                    sp = pspool.tile([128, 2 * TC], FP, tag="s", bufs=2)
                    for h in range(2):
                        nc.tensor.matmul(
                            sp[:, h * TC:(h + 1) * TC],
                            KT[h * 64:(h + 1) * 64, kg * 128:(kg + 1) * 128],
                            QT[h * 64:(h + 1) * 64, t0:t0 + TC],
                            start=True, stop=True,
                        )
                    pt = sbpool.tile([128, 2 * TC], BF, tag="pt",
                                     bufs=AV_DELAY + 4)
                    nc.scalar.activation(pt[:], sp[:], ACT.Exp, scale=0.125)
                    if kt >= 4 * qc:
                        v = kt - 4 * qc
                        nc.vector.tensor_mul(
                            pt[:], pt[:],
                            masks_sb[:, v * 1024:(v + 1) * 1024],
                        )
                    pts[kt] = pt
                    for u in slots.get(kt, ()):
                        u()
                        emitted[0] += 1
                    if kt >= AV_DELAY:
                        emit_av(kt - AV_DELAY)
                for u in tail[emitted[0]:]:
                    u()

                my_tail = []
                for j in range(max(n_kt - AV_DELAY, 0), n_kt):
                    my_tail.append(lambda j=j: emit_av(j))

                bc_cell, bcs_cell, ots_cell = [None], [None], [None]

                def st0():
                    rcsum = rc_tiles[tcx % 2]
                    nc.vector.tensor_copy(rcsum[0:1, :], otp[64:65, 0:TC])
                    nc.vector.tensor_copy(rcsum[32:33, :],
                                          otp[64:65, TC:2 * TC])
                    bc = pspool.tile([128, TC], FP, tag="y", bufs=2,
                                     name="bc")
                    nc.tensor.matmul(bc[:], onesr_sb[:], rc_tiles[tcx % 2][:],
                                     start=True, stop=True)
                    bc_cell[0] = bc

                def st1():
                    bcs = sbpool.tile([128, TC], FP, tag="bcs", bufs=2,
                                      name="bcs")
                    nc.vector.reciprocal_approx_fast(out=bcs[:],
                                                     in_=bc_cell[0][:])
                    bcs_cell[0] = bcs

                def st2():
                    bcs = bcs_cell[0]
                    ots = sbpool.tile([128, TC], BF, tag="ot", bufs=2,
                                      name="ot")
                    nc.vector.tensor_mul(ots[0:64, :], otp[0:64, 0:TC],
                                         bcs[0:64, :])
                    nc.vector.tensor_mul(ots[64:128, :], otp[0:64, TC:2 * TC],
                                         bcs[64:128, :])
                    ots_cell[0] = ots

                ys_cell = [None]

                def proj(half):
                    # out-projection (full 128-contract per m-tile), y DMA
                    # per 2048-col half, queues alternated
                    ots = ots_cell[0]
                    if half == 0:
                        ys_cell[0] = sbpool.tile([128, 8 * TC], BF, tag="ys",
                                                 bufs=2, name="ys")
                    ys = ys_cell[0]
                    for m in range(4 * half, 4 * half + 4):
                        yp = pspool.tile([128, TC], FP, tag="y", bufs=2)
                        nc.tensor.matmul(
                            yp[:], wout_sb[:, m * 128:(m + 1) * 128], ots[:],
                            start=True, stop=True,
                        )
                        if m % 4 == 0:
                            nc.scalar.copy(ys[:, m * TC:(m + 1) * TC], yp[:])
                        else:
                            nc.vector.tensor_copy(
                                ys[:, m * TC:(m + 1) * TC], yp[:])
                    eng = nc.sync if (tcx + half) % 2 == 0 else nc.scalar
                    eng.dma_start(
                        out=ypt[tcx * 128:(tcx + 1) * 128,
                                half * 4 * TC:(half + 1) * 4 * TC],
                        in_=ys[:, half * 4 * TC:(half + 1) * 4 * TC],
                    )

                my_tail += [st0, st1, st2,
                            lambda: proj(0), lambda: proj(1)]
                return my_tail

            # ---- schedule ----
            # Just-in-time QKV pairs: chunks 0-1 only touch tokens of
            # pair 0, chunks 2-3 pairs 0-1, chunks 4-5 pair 2, chunks 6-7
            # pairs 2-3 — so each pair's x-DMA hides behind two Act-paced
            # attention chunks, and each QKV pair (pure PE work) digests
            # an attention tail. Remaining tails ride inside the next
            # chunk's early k-tiles.
            phase_a(0)
            tail = attn(0)
            tail = attn(1, tail)
            phase_a(1, tail)
            tail = attn(2)
            tail = attn(3, tail)
            phase_a(2, tail)
            tail = attn(4)
            tail = attn(5, tail)
            phase_a(3, tail)
            tail = attn(6)
            tail = attn(7, tail)
            for u in tail:
                u()

    nc.compile()
    return nc


def make_in_maps(x, w_qkv, b_qkv):
    x = np.ascontiguousarray(np.asarray(x, np.float32).reshape(BT, C))
    xT = x.T.astype(ml_dtypes.bfloat16)          # [C, BT]
    # xp[pr*128+p, k*1024+q] = xT[k*128+p, pr*1024+q]
    xpack = np.ascontiguousarray(
        xT.reshape(8, 128, 4, 1024).transpose(2, 1, 0, 3).reshape(512, 8192)
    )
    w_qkv = np.asarray(w_qkv, np.float32)
    b_qkv = np.asarray(b_qkv, np.float32)

    onesr = np.zeros((33, 128), np.float32)
    onesr[0, 0:64] = 1.0
    onesr[32, 64:128] = 1.0

    in_maps = []
    for c in range(NCORES):
        sl = slice(c * 128, (c + 1) * 128)
        wcs = np.stack(
            [w_qkv[:, sl], w_qkv[:, 1024:][:, sl], w_qkv[:, 2048:][:, sl]],
            axis=1,
        )  # [1024, 3, 128]
        # wcp[p, k*384 + g*128 + cc] = wcs[k*128 + p, g, cc]
        wcp = np.ascontiguousarray(
            wcs.reshape(8, 128, 3, 128).transpose(1, 0, 2, 3).reshape(128, 3072)
        )
        bq = np.stack(
            [b_qkv[sl], b_qkv[1024:][sl], b_qkv[2048:][sl]], axis=1
        )
        in_maps.append({
            "xp": xpack,
            "wcp": wcp.astype(ml_dtypes.bfloat16),
            "wout": None,  # filled by caller (needs w_out)
            "bqkv": np.ascontiguousarray(bq),
            "onesr": onesr,
        })
    return in_maps


_NC_CACHE = None


def kernel(x, w_qkv, b_qkv, w_out, b_out):
    global _NC_CACHE, LAST_RESULTS
    if _NC_CACHE is None:
        _NC_CACHE = build_nc()
    nc = _NC_CACHE

    w_out = np.asarray(w_out, np.float32)
    in_maps = make_in_maps(x, w_qkv, b_qkv)
    for c in range(NCORES):
        in_maps[c]["wout"] = np.ascontiguousarray(
            w_out[c * 128:(c + 1) * 128, :]).astype(ml_dtypes.bfloat16)

    res = run_bass_kernel_spmd(
        nc, in_maps, list(range(NCORES)),
        trace=bool(os.environ.get("BASS_TRACE")),
    )
    LAST_RESULTS = res

    acc = np.zeros((C, BT), np.float32)
    for out_map in res.results:
        # ypt[tcx*128 + p, m*512 + q] -> y[m*128 + p, tcx*512 + q]
        yp = out_map["ypt"].astype(np.float32)
        acc += yp.reshape(8, 128, 8, 512).transpose(2, 1, 0, 3).reshape(
            C, BT)
    y = acc.T + np.asarray(b_out, np.float32)[None, :]
    return y.reshape(B, T, C)
